# revision 3
# baseline (speedup 1.0000x reference)
"""Trainium2 Bass kernel for nn_ConvAConnect (per-sample-weight 3x3 conv).

Pure data parallel: 16 samples per core on 8 cores. Per (sample, kh) the
conv maps to PE matmuls via a block-Toeplitz weight matrix (lhsT [K=128:
16 in-pixels x 8 cin, M=112: 14 out-pixels x 8 cout]); the input ships
host-transposed to [(x*8+ci), y] strip layout in bf16, the output is
written in [(xo*8+co), strip, y] bf16 and un-permuted/cast on the host.
Bias (bias*Berr) is fused into the PSUM->SBUF copy on the ACT/DVE engines.

v2 over the 122.6us baseline:
- 18 main strips (xo 0..251) + a packed tail: 2 samples per matmul
  (K=2x48 rows, M=2x32) for xo 252..255 instead of a 16th full strip
  (was 16 passes for 4/14 useful columns).
- x-halo dedup: strips 1..17 ship 112 rows; the 16-row overlap is
  rebuilt with an SBUF->SBUF DMA on the otherwise-idle qActDynamicHW
  queue. Saves ~3MB/core of HBM.
- wtile loads on the sync queue, xs split per matmul group so group-A
  strips arrive first; dummy matmuls warm the PE clock (HAM) during
  the DMA lead-in.
"""

import os
import sys

import numpy as np

for _p in ("/opt/trn_rl_repo", "/root/.axon_site/_ro/trn_rl_repo"):
    if os.path.isdir(_p) and _p not in sys.path:
        sys.path.insert(0, _p)

B, H, W_IMG, CIN = 128, 256, 256, 8
KH, KW, COUT = 3, 3, 8

NCORES = 8
BPC = B // NCORES
NPIX = 14
SW = NPIX * COUT
NSM = 18            # main strips, xo 0..251
Y = H
YP = Y + 2
KP = 128
NP2 = BPC // 2      # tail sample-pairs per core
TK = 96             # tail contraction rows (2 samples x 6 xi x 8 ci)
TM = 64             # tail out rows (2 samples x 4 xo x 8 co)

TRACE = False
LAST_RESULT = [None]

_NC_CACHE = [None]


def _build_nc():
    import concourse.bass as bass
    import concourse.mybir as mybir
    from concourse.tile import TileContext

    f32 = mybir.dt.float32
    bf16 = mybir.dt.bfloat16
    nc = bass.Bass()
    xsa = nc.declare_dram_parameter("xsa", [BPC, KP, YP], bf16, isOutput=False)
    xsb = nc.declare_dram_parameter("xsb", [BPC, 112, NSM - 1, YP], bf16,
                                    isOutput=False)
    xst = nc.declare_dram_parameter("xst", [NP2, TK, YP], bf16, isOutput=False)
    tw = nc.declare_dram_parameter("tw", [BPC, KP, KH, SW], bf16, isOutput=False)
    twt = nc.declare_dram_parameter("twt", [NP2, TK, KH, TM], bf16,
                                    isOutput=False)
    bi = nc.declare_dram_parameter("bi", [SW, BPC], f32, isOutput=False)
    bit = nc.declare_dram_parameter("bit", [TM, NP2], f32, isOutput=False)
    zt = nc.declare_dram_parameter("zt", [BPC, SW, NSM, Y], bf16, isOutput=True)
    ztt = nc.declare_dram_parameter("ztt", [NP2, TM, Y], bf16, isOutput=True)

    # 9 two-strip units; group A = strips 0..9, group B = strips 10..17
    units = [(s, 2) for s in range(0, NSM, 2)]
    groups = [units[:5], units[5:]]

    with TileContext(nc) as tc:
        with (
            tc.tile_pool(name="xp", bufs=4) as xp,
            tc.tile_pool(name="wp", bufs=3) as wp,
            tc.tile_pool(name="op", bufs=4) as op,
            tc.tile_pool(name="bp", bufs=1) as bp,
            tc.tile_pool(name="tp", bufs=2) as tpp,
            tc.tile_pool(name="pp", bufs=7, space="PSUM") as pp,
            tc.tile_pool(name="tq", bufs=1, space="PSUM") as tq,
        ):
            bias_t = bp.tile([SW, BPC], f32, tag="bias")
            nc.sync.dma_start(out=bias_t, in_=bi[:, :])
            bias_tt = bp.tile([TM, NP2], f32, tag="biast")
            nc.sync.dma_start(out=bias_tt, in_=bit[:, :])

            # PE warm-up: junk matmuls during the DMA lead-in so the HAM
            # clock gate flips to 2.4GHz before real work arrives.
            junk = bp.tile([TK, 576], bf16, tag="junk")
            nc.vector.memset(junk, 0.0)
            pj = tq.tile([TK, 2 * Y], f32, name="pt", tag="pt")
            NDUMMY = 14
            for i in range(NDUMMY):
                nc.tensor.matmul(
                    out=pj[0:TM, 0:Y],
                    lhsT=junk[0:TK, 0:TM],
                    rhs=junk[0:TK, 320:320 + Y],
                    start=(i == 0),
                    stop=(i == NDUMMY - 1),
                )

            for b in range(BPC):
                wtile = wp.tile([KP, KH, SW], bf16)
                nc.sync.dma_start(out=wtile, in_=tw[b])
                xtile = xp.tile([KP, NSM, YP], bf16)
                # strip 0 full 128 rows
                nc.gpsimd.dma_start(out=xtile[:, 0:1, :], in_=xsa[b])
                # strips 1..9 rows 16..127 (group A), then 10..17 (group B)
                nc.gpsimd.dma_start(out=xtile[16:128, 1:10, :],
                                    in_=xsb[b, :, 0:9, :])
                nc.gpsimd.dma_start(out=xtile[16:128, 10:NSM, :],
                                    in_=xsb[b, :, 9:NSM - 1, :])
                # halo: strip s rows 0..15 = strip s-1 rows 112..127
                nc.scalar.dma_start(out=xtile[0:16, 1:11, :],
                                    in_=xtile[112:128, 0:10, :])
                nc.scalar.dma_start(out=xtile[0:16, 11:NSM, :],
                                    in_=xtile[112:128, 10:NSM - 1, :])

                nadd = 0
                for grp in groups:
                    g0 = grp[0][0]
                    nstrips = sum(w for _, w in grp)
                    pts = [pp.tile([128, 2 * Y], f32, name="pt", tag="pt")
                           for _ in grp]
                    otile = op.tile([SW, nstrips * Y], bf16,
                                    tag=f"ot{nstrips}")
                    for kh in range(KH):
                        lhsT = wtile[:, kh, :]
                        for j, (s, w) in enumerate(grp):
                            nc.tensor.matmul(
                                out=pts[j][0:SW, 0:w * Y],
                                lhsT=lhsT,
                                rhs=xtile[:, s:s + w, kh:kh + Y],
                                start=(kh == 0),
                                stop=(kh == KH - 1),
                            )
                    for j, (s, w) in enumerate(grp):
                        c0 = (s - g0) * Y
                        if nadd % 2 == 0:
                            nc.vector.tensor_scalar_add(
                                out=otile[0:SW, c0:c0 + w * Y],
                                in0=pts[j][0:SW, 0:w * Y],
                                scalar1=bias_t[:, b:b + 1],
                            )
                        else:
                            nc.scalar.add(
                                out=otile[0:SW, c0:c0 + w * Y],
                                in_=pts[j][0:SW, 0:w * Y],
                                add=bias_t[:, b:b + 1],
                            )
                        nadd += 1
                    nc.sync.dma_start(
                        out=zt[b, :, g0:g0 + nstrips, :],
                        in_=otile[0:SW, 0:nstrips * Y],
                    )

                # tail: xo 252..255 for the sample pair (b-1, b)
                if b % 2 == 1:
                    g = b // 2
                    xtt = tpp.tile([TK, YP], bf16, tag="xtt")
                    nc.gpsimd.dma_start(out=xtt, in_=xst[g])
                    twt_t = tpp.tile([TK, KH, TM], bf16, tag="twt")
                    nc.sync.dma_start(out=twt_t, in_=twt[g])
                    ptt = tq.tile([TK, 2 * Y], f32, name="pt", tag="pt")
                    for kh in range(KH):
                        nc.tensor.matmul(
                            out=ptt[0:TM, 0:Y],
                            lhsT=twt_t[:, kh, :],
                            rhs=xtt[:, kh:kh + Y],
                            start=(kh == 0),
                            stop=(kh == KH - 1),
                        )
                    ot2 = tpp.tile([TM, Y], bf16, tag="ot2")
                    if g % 2 == 0:
                        nc.vector.tensor_scalar_add(
                            out=ot2, in0=ptt[0:TM, 0:Y],
                            scalar1=bias_tt[:, g:g + 1],
                        )
                    else:
                        nc.scalar.add(
                            out=ot2, in_=ptt[0:TM, 0:Y],
                            add=bias_tt[:, g:g + 1],
                        )
                    nc.sync.dma_start(out=ztt[g], in_=ot2)
    _split_multi_waits(nc, mybir)
    return nc


def _split_multi_waits(nc, mybir):
    nid = [0]
    for fn in nc.m.functions:
        for blk in fn.blocks:
            out = []
            for inst in blk.instructions:
                si = inst.sync_info
                if si is not None and si.on_wait and len(si.on_wait) > 1:
                    waits = list(si.on_wait)
                    for w in waits[:-1]:
                        nid[0] += 1
                        out.append(mybir.InstNoOp(
                            name=f"nopw-{nid[0]}",
                            engine=inst.engine,
                            ins=[],
                            outs=[],
                            sync_info=mybir.SyncInfo(on_wait=[w], on_update=[]),
                        ))
                    inst.sync_info = mybir.SyncInfo(
                        on_wait=[waits[-1]],
                        on_update=list(si.on_update or []),
                    )
                out.append(inst)
            blk.instructions[:] = out


def _get_nc():
    if _NC_CACHE[0] is None:
        _NC_CACHE[0] = _build_nc()
    return _NC_CACHE[0]


def host_prep(X, W, bias, Werr, Berr):
    X = np.asarray(X, np.float32)
    W = np.asarray(W, np.float32)
    bias = np.asarray(bias, np.float32)
    Werr = np.asarray(Werr, np.float32)
    Berr = np.asarray(Berr, np.float32)

    memW = W[None] * Werr                      # [B, kh, kw, cin, cout]
    TW = np.zeros((B, KP, KH, SW), np.float32)
    for kw in range(KW):
        blk = memW[:, :, kw].transpose(0, 2, 1, 3)   # [B, ci, kh, co]
        for xo in range(NPIX):
            xi = xo + kw
            TW[:, xi * 8:(xi + 1) * 8, :, xo * 8:(xo + 1) * 8] = blk

    # tail lhsT: block-diag over the 2 samples of each pair
    TWT = np.zeros((B // 2, TK, KH, TM), np.float32)
    for j in range(2):
        blkj = (memW[j::2]).transpose(0, 1, 3, 4, 2)  # [B/2, kh, ci, co, kw]
        for kw in range(KW):
            blk = blkj[..., kw].transpose(0, 2, 1, 3)  # [B/2, ci, kh, co]
            for xo in range(4):
                xi = xo + kw
                TWT[:, 48 * j + 8 * xi:48 * j + 8 * (xi + 1), :,
                    32 * j + 8 * xo:32 * j + 8 * (xo + 1)] = blk

    BIT = np.tile(bias[None] * Berr, (1, NPIX))        # [B, SW]
    membias = bias[None] * Berr                        # [B, COUT]
    BIT2 = np.zeros((B // 2, TM), np.float32)
    for j in range(2):
        BIT2[:, 32 * j:32 * (j + 1)] = np.tile(membias[j::2], (1, 4))

    import ml_dtypes
    bf16 = ml_dtypes.bfloat16
    XTP = np.zeros((B, 2176, YP), bf16)
    XTP[:, 8:8 + W_IMG * CIN, 1:1 + Y] = \
        X.transpose(0, 2, 3, 1).reshape(B, W_IMG * CIN, Y)
    XSA = np.ascontiguousarray(XTP[:, 0:128])                 # [B,128,YP]
    XSB = np.ascontiguousarray(
        XTP[:, 128:128 + 112 * (NSM - 1)]
        .reshape(B, NSM - 1, 112, YP).transpose(0, 2, 1, 3))  # [B,112,17,YP]
    # tail rows: xi 251..256 -> XT rows 2016..2063, paired samples
    XST = np.ascontiguousarray(
        XTP[:, 2016:2064].reshape(B // 2, 2 * 48, YP))        # [B/2,96,YP]
    return XSA, XSB, XST, TW.astype(bf16), TWT.astype(bf16), BIT, BIT2


def host_unpack(zt_all, ztt_all):
    out = np.empty((B, Y, W_IMG, COUT), np.float32)
    z = zt_all.reshape(B, NPIX, COUT, NSM, Y)
    z = z.transpose(0, 4, 3, 1, 2).reshape(B, Y, NSM * NPIX, COUT)
    out[:, :, :NSM * NPIX, :] = z
    zt2 = ztt_all.reshape(B // 2, 2, 4, COUT, Y)   # [pair, j, xo, co, y]
    for j in range(2):
        out[j::2, :, NSM * NPIX:, :] = zt2[:, j].transpose(0, 3, 1, 2)
    return out


def kernel(X, W, bias, Werr, Berr):
    from concourse.bass_utils import run_bass_kernel_spmd

    XSA, XSB, XST, TW, TWT, BIT, BIT2 = host_prep(X, W, bias, Werr, Berr)
    in_maps = []
    for m in range(NCORES):
        sl = slice(m * BPC, (m + 1) * BPC)
        sl2 = slice(m * NP2, (m + 1) * NP2)
        in_maps.append({
            "xsa": np.ascontiguousarray(XSA[sl]),
            "xsb": np.ascontiguousarray(XSB[sl]),
            "xst": np.ascontiguousarray(XST[sl2]),
            "tw": np.ascontiguousarray(TW[sl]),
            "twt": np.ascontiguousarray(TWT[sl2]),
            "bi": np.ascontiguousarray(BIT[sl].T),
            "bit": np.ascontiguousarray(BIT2[sl2].T),
        })
    nc = _get_nc()
    res = run_bass_kernel_spmd(nc, in_maps, core_ids=list(range(NCORES)), trace=TRACE)
    LAST_RESULT[0] = res
    zt_all = np.concatenate([r["zt"] for r in res.results], axis=0)
    ztt_all = np.concatenate([r["ztt"] for r in res.results], axis=0)
    return host_unpack(zt_all, ztt_all)


# revision 4
# speedup vs baseline: 1.0123x; 1.0123x over previous
"""Trainium2 Bass kernel for nn_ConvAConnect (per-sample-weight 3x3 conv).

Pure data parallel: 16 samples per core on 8 cores. Per (sample, kh) the
conv maps to PE matmuls via a block-Toeplitz weight matrix (lhsT [K=128:
16 in-pixels x 8 cin, M=112: 14 out-pixels x 8 cout]); the input ships
host-transposed to [(x*8+ci), y] strip layout in bf16, the output is
written in [(xo*8+co), strip, y] bf16 and un-permuted/cast on the host.
Bias (bias*Berr) is fused into the PSUM->SBUF copy on the ACT/DVE engines.

v3 over the 122.6us baseline:
- 18 main strips (xo 0..251) + a packed tail: 2 samples per matmul
  (K=2x48 rows, M=2x32) for xo 252..255 instead of a 16th full strip
  (was 16 full 256-column passes for 4/14 useful output columns).
  Saves ~2.5us PE and ~1MB/core HBM.
- y-pad columns (y=-1, y=256) are zeroed once in a manual 4-tile ring
  instead of being shipped; xs rows are 512B.
- Dummy matmuls warm the PE clock (HAM) during the DMA lead-in; the
  first sample's xs ships as two DMAs so group-A strips arrive first.
- Tail-pair inputs are prefetched on the input (gpsimd) queue one
  sample ahead so they never sit behind output DMAs.

Keeping (from the baseline): SBUF->SBUF halo reconstruction and any
third concurrent DMA queue measurably regress (per-DMA ~1us floor +
packet round-robin starves the two main streams) -- the x-halo stays
in HBM.
"""

import os
import sys

import numpy as np

for _p in ("/opt/trn_rl_repo", "/root/.axon_site/_ro/trn_rl_repo"):
    if os.path.isdir(_p) and _p not in sys.path:
        sys.path.insert(0, _p)

B, H, W_IMG, CIN = 128, 256, 256, 8
KH, KW, COUT = 3, 3, 8

NCORES = 8
BPC = B // NCORES
NPIX = 14
SW = NPIX * COUT
NSM = 18            # main strips, xo 0..251
Y = H
YP = Y + 2
KP = 128
NP2 = BPC // 2      # tail sample-pairs per core
TK = 96             # tail contraction rows (2 samples x 6 xi x 8 ci)
TM = 64             # tail out rows (2 samples x 4 xo x 8 co)

TRACE = False
LAST_RESULT = [None]

_NC_CACHE = [None]


def _build_nc():
    import concourse.bass as bass
    import concourse.mybir as mybir
    from concourse.tile import TileContext

    f32 = mybir.dt.float32
    bf16 = mybir.dt.bfloat16
    nc = bass.Bass()
    xs = nc.declare_dram_parameter("xs", [BPC, KP, NSM, Y], bf16, isOutput=False)
    xst = nc.declare_dram_parameter("xst", [NP2, TK, YP], bf16, isOutput=False)
    tw = nc.declare_dram_parameter("tw", [BPC, KP, KH, SW], bf16, isOutput=False)
    twt = nc.declare_dram_parameter("twt", [NP2, TK, KH, TM], bf16,
                                    isOutput=False)
    bi = nc.declare_dram_parameter("bi", [SW, BPC], f32, isOutput=False)
    bit = nc.declare_dram_parameter("bit", [TM, NP2], f32, isOutput=False)
    zt = nc.declare_dram_parameter("zt", [BPC, SW, NSM, Y], bf16, isOutput=True)
    ztt = nc.declare_dram_parameter("ztt", [NP2, TM, Y], bf16, isOutput=True)

    # 9 two-strip units; group A = strips 0..9, group B = strips 10..17
    units = [(s, 2) for s in range(0, NSM, 2)]
    groups = [units[:5], units[5:]]

    with TileContext(nc) as tc:
        with (
            tc.tile_pool(name="xp", bufs=1) as xp,
            tc.tile_pool(name="wp", bufs=3) as wp,
            tc.tile_pool(name="op", bufs=4) as op,
            tc.tile_pool(name="bp", bufs=1) as bp,
            tc.tile_pool(name="tp", bufs=2) as tpp,
            tc.tile_pool(name="pp", bufs=7, space="PSUM") as pp,
            tc.tile_pool(name="tq", bufs=1, space="PSUM") as tq,
        ):
            bias_t = bp.tile([SW, BPC], f32, tag="bias")
            nc.sync.dma_start(out=bias_t, in_=bi[:, :])
            bias_tt = bp.tile([TM, NP2], f32, tag="biast")
            nc.sync.dma_start(out=bias_tt, in_=bit[:, :])

            # manual 4-deep xtile ring with the y-pad columns zeroed once
            xring = []
            for r in range(4):
                xt = xp.tile([KP, NSM, YP], bf16, name=f"xr{r}", tag=f"xr{r}")
                nc.vector.memset(xt[:, :, 0:1], 0.0)
                nc.vector.memset(xt[:, :, YP - 1:YP], 0.0)
                xring.append(xt)

            # PE warm-up: junk matmuls during the DMA lead-in so the HAM
            # clock gate flips to 2.4GHz before real work arrives.
            junk = bp.tile([TK, 576], bf16, tag="junk")
            nc.vector.memset(junk, 0.0)
            pj = tq.tile([TK, 2 * Y], f32, name="pt", tag="pt")
            NDUMMY = 10
            for i in range(NDUMMY):
                nc.tensor.matmul(
                    out=pj[0:TM, 0:Y],
                    lhsT=junk[0:TK, 0:TM],
                    rhs=junk[0:TK, 320:320 + Y],
                    start=(i == 0),
                    stop=(i == NDUMMY - 1),
                )

            for b in range(BPC):
                xtile = xring[b % 4]
                wtile = wp.tile([KP, KH, SW], bf16)
                nc.gpsimd.dma_start(out=wtile, in_=tw[b])
                if b == 0:
                    nc.gpsimd.dma_start(out=xtile[:, 0:10, 1:1 + Y],
                                        in_=xs[b, :, 0:10, :])
                    nc.gpsimd.dma_start(out=xtile[:, 10:NSM, 1:1 + Y],
                                        in_=xs[b, :, 10:NSM, :])
                else:
                    nc.gpsimd.dma_start(out=xtile[:, :, 1:1 + Y], in_=xs[b])
                # prefetch next tail pair's inputs on the input queue
                if b % 2 == 0:
                    g = b // 2
                    xtt = tpp.tile([TK, YP], bf16, tag="xtt")
                    nc.gpsimd.dma_start(out=xtt, in_=xst[g])
                    twt_t = tpp.tile([TK, KH, TM], bf16, tag="twt")
                    nc.gpsimd.dma_start(out=twt_t, in_=twt[g])

                nadd = 0
                for grp in groups:
                    g0 = grp[0][0]
                    nstrips = sum(w for _, w in grp)
                    pts = [pp.tile([128, 2 * Y], f32, name="pt", tag="pt")
                           for _ in grp]
                    otile = op.tile([SW, nstrips * Y], bf16,
                                    tag=f"ot{nstrips}")
                    for kh in range(KH):
                        lhsT = wtile[:, kh, :]
                        for j, (s, w) in enumerate(grp):
                            nc.tensor.matmul(
                                out=pts[j][0:SW, 0:w * Y],
                                lhsT=lhsT,
                                rhs=xtile[:, s:s + w, kh:kh + Y],
                                start=(kh == 0),
                                stop=(kh == KH - 1),
                            )
                    for j, (s, w) in enumerate(grp):
                        c0 = (s - g0) * Y
                        if nadd % 2 == 0:
                            nc.vector.tensor_scalar_add(
                                out=otile[0:SW, c0:c0 + w * Y],
                                in0=pts[j][0:SW, 0:w * Y],
                                scalar1=bias_t[:, b:b + 1],
                            )
                        else:
                            nc.scalar.add(
                                out=otile[0:SW, c0:c0 + w * Y],
                                in_=pts[j][0:SW, 0:w * Y],
                                add=bias_t[:, b:b + 1],
                            )
                        nadd += 1
                    nc.sync.dma_start(
                        out=zt[b, :, g0:g0 + nstrips, :],
                        in_=otile[0:SW, 0:nstrips * Y],
                    )

                # tail: xo 252..255 for the sample pair (b-1, b)
                if b % 2 == 1:
                    g = b // 2
                    ptt = tq.tile([TK, 2 * Y], f32, name="pt", tag="pt")
                    for kh in range(KH):
                        nc.tensor.matmul(
                            out=ptt[0:TM, 0:Y],
                            lhsT=twt_t[:, kh, :],
                            rhs=xtt[:, kh:kh + Y],
                            start=(kh == 0),
                            stop=(kh == KH - 1),
                        )
                    ot2 = tpp.tile([TM, Y], bf16, tag="ot2")
                    if g % 2 == 0:
                        nc.vector.tensor_scalar_add(
                            out=ot2, in0=ptt[0:TM, 0:Y],
                            scalar1=bias_tt[:, g:g + 1],
                        )
                    else:
                        nc.scalar.add(
                            out=ot2, in_=ptt[0:TM, 0:Y],
                            add=bias_tt[:, g:g + 1],
                        )
                    nc.sync.dma_start(out=ztt[g], in_=ot2)
    _split_multi_waits(nc, mybir)
    return nc


def _split_multi_waits(nc, mybir):
    nid = [0]
    for fn in nc.m.functions:
        for blk in fn.blocks:
            out = []
            for inst in blk.instructions:
                si = inst.sync_info
                if si is not None and si.on_wait and len(si.on_wait) > 1:
                    waits = list(si.on_wait)
                    for w in waits[:-1]:
                        nid[0] += 1
                        out.append(mybir.InstNoOp(
                            name=f"nopw-{nid[0]}",
                            engine=inst.engine,
                            ins=[],
                            outs=[],
                            sync_info=mybir.SyncInfo(on_wait=[w], on_update=[]),
                        ))
                    inst.sync_info = mybir.SyncInfo(
                        on_wait=[waits[-1]],
                        on_update=list(si.on_update or []),
                    )
                out.append(inst)
            blk.instructions[:] = out


def _get_nc():
    if _NC_CACHE[0] is None:
        _NC_CACHE[0] = _build_nc()
    return _NC_CACHE[0]


def host_prep(X, W, bias, Werr, Berr):
    X = np.asarray(X, np.float32)
    W = np.asarray(W, np.float32)
    bias = np.asarray(bias, np.float32)
    Werr = np.asarray(Werr, np.float32)
    Berr = np.asarray(Berr, np.float32)

    memW = W[None] * Werr                      # [B, kh, kw, cin, cout]
    TW = np.zeros((B, KP, KH, SW), np.float32)
    for kw in range(KW):
        blk = memW[:, :, kw].transpose(0, 2, 1, 3)   # [B, ci, kh, co]
        for xo in range(NPIX):
            xi = xo + kw
            TW[:, xi * 8:(xi + 1) * 8, :, xo * 8:(xo + 1) * 8] = blk

    # tail lhsT: block-diag over the 2 samples of each pair
    TWT = np.zeros((B // 2, TK, KH, TM), np.float32)
    for j in range(2):
        blkj = (memW[j::2]).transpose(0, 1, 3, 4, 2)  # [B/2, kh, ci, co, kw]
        for kw in range(KW):
            blk = blkj[..., kw].transpose(0, 2, 1, 3)  # [B/2, ci, kh, co]
            for xo in range(4):
                xi = xo + kw
                TWT[:, 48 * j + 8 * xi:48 * j + 8 * (xi + 1), :,
                    32 * j + 8 * xo:32 * j + 8 * (xo + 1)] = blk

    BIT = np.tile(bias[None] * Berr, (1, NPIX))        # [B, SW]
    membias = bias[None] * Berr                        # [B, COUT]
    BIT2 = np.zeros((B // 2, TM), np.float32)
    for j in range(2):
        BIT2[:, 32 * j:32 * (j + 1)] = np.tile(membias[j::2], (1, 4))

    import ml_dtypes
    bf16 = ml_dtypes.bfloat16
    XTP = np.zeros((B, 2176, YP), bf16)
    XTP[:, 8:8 + W_IMG * CIN, 1:1 + Y] = \
        X.transpose(0, 2, 3, 1).reshape(B, W_IMG * CIN, Y)
    # main strips: whole 128 rows, y-pad columns dropped (zeroed on chip)
    XS = np.empty((B, KP, NSM, Y), bf16)
    for s in range(NSM):
        XS[:, :, s, :] = XTP[:, 112 * s:112 * s + KP, 1:1 + Y]
    # tail rows: xi 251..256 -> XT rows 2016..2063, paired samples
    XST = np.ascontiguousarray(
        XTP[:, 2016:2064].reshape(B // 2, 2 * 48, YP))        # [B/2,96,YP]
    return XS, XST, TW.astype(bf16), TWT.astype(bf16), BIT, BIT2


def host_unpack(zt_all, ztt_all):
    out = np.empty((B, Y, W_IMG, COUT), np.float32)
    z = zt_all.reshape(B, NPIX, COUT, NSM, Y)
    z = z.transpose(0, 4, 3, 1, 2).reshape(B, Y, NSM * NPIX, COUT)
    out[:, :, :NSM * NPIX, :] = z
    zt2 = ztt_all.reshape(B // 2, 2, 4, COUT, Y)   # [pair, j, xo, co, y]
    for j in range(2):
        out[j::2, :, NSM * NPIX:, :] = zt2[:, j].transpose(0, 3, 1, 2)
    return out


def kernel(X, W, bias, Werr, Berr):
    from concourse.bass_utils import run_bass_kernel_spmd

    XS, XST, TW, TWT, BIT, BIT2 = host_prep(X, W, bias, Werr, Berr)
    in_maps = []
    for m in range(NCORES):
        sl = slice(m * BPC, (m + 1) * BPC)
        sl2 = slice(m * NP2, (m + 1) * NP2)
        in_maps.append({
            "xs": np.ascontiguousarray(XS[sl]),
            "xst": np.ascontiguousarray(XST[sl2]),
            "tw": np.ascontiguousarray(TW[sl]),
            "twt": np.ascontiguousarray(TWT[sl2]),
            "bi": np.ascontiguousarray(BIT[sl].T),
            "bit": np.ascontiguousarray(BIT2[sl2].T),
        })
    nc = _get_nc()
    res = run_bass_kernel_spmd(nc, in_maps, core_ids=list(range(NCORES)), trace=TRACE)
    LAST_RESULT[0] = res
    zt_all = np.concatenate([r["zt"] for r in res.results], axis=0)
    ztt_all = np.concatenate([r["ztt"] for r in res.results], axis=0)
    return host_unpack(zt_all, ztt_all)


# revision 9
# speedup vs baseline: 1.1385x; 1.1246x over previous
"""Trainium2 Bass kernel for nn_ConvAConnect (per-sample-weight 3x3 conv).

Pure data parallel: 16 samples per core on 8 cores. Per (sample, kh) the
conv maps to PE matmuls via a block-Toeplitz weight matrix (lhsT [K=128:
16 in-pixels x 8 cin, M=112: 14 out-pixels x 8 cout]); the input ships
host-transposed to [(x*8+ci), y] strip layout in bf16, the output is
written in [(xo*8+co), strip, y] bf16 and un-permuted/cast on the host.
Bias (bias*Berr) is fused into the PSUM->SBUF copy on the ACT/DVE engines.

v3 over the 122.6us baseline:
- 18 main strips (xo 0..251) + a packed tail: 2 samples per matmul
  (K=2x48 rows, M=2x32) for xo 252..255 instead of a 16th full strip
  (was 16 full 256-column passes for 4/14 useful output columns).
  Saves ~2.5us PE and ~1MB/core HBM.
- y-pad columns (y=-1, y=256) are zeroed once in a manual 4-tile ring
  instead of being shipped; xs rows are 512B.
- Dummy matmuls warm the PE clock (HAM) during the DMA lead-in; the
  first sample's xs ships as two DMAs so group-A strips arrive first.
- Tail-pair inputs are prefetched on the input (gpsimd) queue one
  sample ahead so they never sit behind output DMAs.

Keeping (from the baseline): SBUF->SBUF halo reconstruction and any
third concurrent DMA queue measurably regress (per-DMA ~1us floor +
packet round-robin starves the two main streams) -- the x-halo stays
in HBM.
"""

import os
import sys

import numpy as np

for _p in ("/opt/trn_rl_repo", "/root/.axon_site/_ro/trn_rl_repo"):
    if os.path.isdir(_p) and _p not in sys.path:
        sys.path.insert(0, _p)

B, H, W_IMG, CIN = 128, 256, 256, 8
KH, KW, COUT = 3, 3, 8

NCORES = 8
BPC = B // NCORES
NPIX = 14
SW = NPIX * COUT
NSM = 18            # main strips, xo 0..251
Y = H
YP = Y + 2
KP = 128
NP2 = BPC // 2      # tail sample-pairs per core
TK = 96             # tail contraction rows (2 samples x 6 xi x 8 ci)
TM = 64             # tail out rows (2 samples x 4 xo x 8 co)

TRACE = False
LAST_RESULT = [None]

_NC_CACHE = [None]


def _build_nc():
    import concourse.bass as bass
    import concourse.mybir as mybir
    from concourse.tile import TileContext

    f32 = mybir.dt.float32
    bf16 = mybir.dt.bfloat16
    nc = bass.Bass()
    xs = nc.declare_dram_parameter("xs", [BPC, KP, NSM, YP], bf16, isOutput=False)
    xst = nc.declare_dram_parameter("xst", [NP2, TK, YP], bf16, isOutput=False)
    tw = nc.declare_dram_parameter("tw", [BPC, KP, KH, SW], bf16, isOutput=False)
    twt = nc.declare_dram_parameter("twt", [NP2, TK, KH, TM], bf16,
                                    isOutput=False)
    bi = nc.declare_dram_parameter("bi", [SW, BPC], f32, isOutput=False)
    bit = nc.declare_dram_parameter("bit", [TM, NP2], f32, isOutput=False)
    zt = nc.declare_dram_parameter("zt", [BPC, SW, NSM, Y], bf16, isOutput=True)
    ztt = nc.declare_dram_parameter("ztt", [NP2, TM, Y], bf16, isOutput=True)

    # 9 two-strip units; group A = strips 0..9, group B = strips 10..17
    units = [(s, 2) for s in range(0, NSM, 2)]
    groups = [units[:5], units[5:]]

    with TileContext(nc) as tc:
        with (
            tc.tile_pool(name="xp", bufs=4) as xp,
            tc.tile_pool(name="wp", bufs=3) as wp,
            tc.tile_pool(name="op", bufs=4) as op,
            tc.tile_pool(name="bp", bufs=1) as bp,
            tc.tile_pool(name="tp", bufs=2) as tpp,
            tc.tile_pool(name="pp", bufs=7, space="PSUM") as pp,
            tc.tile_pool(name="tq", bufs=1, space="PSUM") as tq,
        ):
            bias_t = bp.tile([SW, BPC], f32, tag="bias")
            nc.sync.dma_start(out=bias_t, in_=bi[:, :])
            bias_tt = bp.tile([TM, NP2], f32, tag="biast")
            nc.sync.dma_start(out=bias_tt, in_=bit[:, :])

            # PE warm-up: junk matmuls during the DMA lead-in so the HAM
            # clock gate flips to 2.4GHz before real work arrives.
            junk = bp.tile([TK, 576], bf16, tag="junk")
            nc.vector.memset(junk, 0.0)
            pj = tq.tile([TK, 2 * Y], f32, name="pt", tag="pt")
            NDUMMY = 10
            for i in range(NDUMMY):
                nc.tensor.matmul(
                    out=pj[0:TM, 0:Y],
                    lhsT=junk[0:TK, 0:TM],
                    rhs=junk[0:TK, 320:320 + Y],
                    start=(i == 0),
                    stop=(i == NDUMMY - 1),
                )

            for b in range(BPC):
                xtile = xp.tile([KP, NSM, YP], bf16)
                wtile = wp.tile([KP, KH, SW], bf16)
                nc.gpsimd.dma_start(out=wtile, in_=tw[b])
                if b == 0:
                    nc.gpsimd.dma_start(out=xtile[:, 0:10, :],
                                        in_=xs[b, :, 0:10, :])
                    nc.gpsimd.dma_start(out=xtile[:, 10:NSM, :],
                                        in_=xs[b, :, 10:NSM, :])
                else:
                    nc.gpsimd.dma_start(out=xtile, in_=xs[b])
                # prefetch next tail pair's inputs on the input queue
                if b % 2 == 0:
                    g = b // 2
                    xtt = tpp.tile([TK, YP], bf16, tag="xtt")
                    nc.gpsimd.dma_start(out=xtt, in_=xst[g])
                    twt_t = tpp.tile([TK, KH, TM], bf16, tag="twt")
                    nc.gpsimd.dma_start(out=twt_t, in_=twt[g])

                nadd = 0
                for grp in groups:
                    g0 = grp[0][0]
                    nstrips = sum(w for _, w in grp)
                    pts = [pp.tile([128, 2 * Y], f32, name="pt", tag="pt")
                           for _ in grp]
                    otile = op.tile([SW, nstrips * Y], bf16,
                                    tag=f"ot{nstrips}")
                    for kh in range(KH):
                        lhsT = wtile[:, kh, :]
                        for j, (s, w) in enumerate(grp):
                            nc.tensor.matmul(
                                out=pts[j][0:SW, 0:w * Y],
                                lhsT=lhsT,
                                rhs=xtile[:, s:s + w, kh:kh + Y],
                                start=(kh == 0),
                                stop=(kh == KH - 1),
                            )
                    for j, (s, w) in enumerate(grp):
                        c0 = (s - g0) * Y
                        if nadd % 2 == 0:
                            nc.vector.tensor_scalar_add(
                                out=otile[0:SW, c0:c0 + w * Y],
                                in0=pts[j][0:SW, 0:w * Y],
                                scalar1=bias_t[:, b:b + 1],
                            )
                        else:
                            nc.scalar.add(
                                out=otile[0:SW, c0:c0 + w * Y],
                                in_=pts[j][0:SW, 0:w * Y],
                                add=bias_t[:, b:b + 1],
                            )
                        nadd += 1
                    nc.sync.dma_start(
                        out=zt[b, :, g0:g0 + nstrips, :],
                        in_=otile[0:SW, 0:nstrips * Y],
                    )

                # tail: xo 252..255 for the sample pair (b-1, b)
                if b % 2 == 1:
                    g = b // 2
                    ptt = tq.tile([TK, 2 * Y], f32, name="pt", tag="pt")
                    for kh in range(KH):
                        nc.tensor.matmul(
                            out=ptt[0:TM, 0:Y],
                            lhsT=twt_t[:, kh, :],
                            rhs=xtt[:, kh:kh + Y],
                            start=(kh == 0),
                            stop=(kh == KH - 1),
                        )
                    ot2 = tpp.tile([TM, Y], bf16, tag="ot2")
                    if g % 2 == 0:
                        nc.vector.tensor_scalar_add(
                            out=ot2, in0=ptt[0:TM, 0:Y],
                            scalar1=bias_tt[:, g:g + 1],
                        )
                    else:
                        nc.scalar.add(
                            out=ot2, in_=ptt[0:TM, 0:Y],
                            add=bias_tt[:, g:g + 1],
                        )
                    nc.sync.dma_start(out=ztt[g], in_=ot2)
    _split_multi_waits(nc, mybir)
    return nc


def _split_multi_waits(nc, mybir):
    nid = [0]
    for fn in nc.m.functions:
        for blk in fn.blocks:
            out = []
            for inst in blk.instructions:
                si = inst.sync_info
                if si is not None and si.on_wait and len(si.on_wait) > 1:
                    waits = list(si.on_wait)
                    for w in waits[:-1]:
                        nid[0] += 1
                        out.append(mybir.InstNoOp(
                            name=f"nopw-{nid[0]}",
                            engine=inst.engine,
                            ins=[],
                            outs=[],
                            sync_info=mybir.SyncInfo(on_wait=[w], on_update=[]),
                        ))
                    inst.sync_info = mybir.SyncInfo(
                        on_wait=[waits[-1]],
                        on_update=list(si.on_update or []),
                    )
                out.append(inst)
            blk.instructions[:] = out


def _get_nc():
    if _NC_CACHE[0] is None:
        _NC_CACHE[0] = _build_nc()
    return _NC_CACHE[0]


def host_prep(X, W, bias, Werr, Berr):
    X = np.asarray(X, np.float32)
    W = np.asarray(W, np.float32)
    bias = np.asarray(bias, np.float32)
    Werr = np.asarray(Werr, np.float32)
    Berr = np.asarray(Berr, np.float32)

    memW = W[None] * Werr                      # [B, kh, kw, cin, cout]
    TW = np.zeros((B, KP, KH, SW), np.float32)
    for kw in range(KW):
        blk = memW[:, :, kw].transpose(0, 2, 1, 3)   # [B, ci, kh, co]
        for xo in range(NPIX):
            xi = xo + kw
            TW[:, xi * 8:(xi + 1) * 8, :, xo * 8:(xo + 1) * 8] = blk

    # tail lhsT: block-diag over the 2 samples of each pair
    TWT = np.zeros((B // 2, TK, KH, TM), np.float32)
    for j in range(2):
        blkj = (memW[j::2]).transpose(0, 1, 3, 4, 2)  # [B/2, kh, ci, co, kw]
        for kw in range(KW):
            blk = blkj[..., kw].transpose(0, 2, 1, 3)  # [B/2, ci, kh, co]
            for xo in range(4):
                xi = xo + kw
                TWT[:, 48 * j + 8 * xi:48 * j + 8 * (xi + 1), :,
                    32 * j + 8 * xo:32 * j + 8 * (xo + 1)] = blk

    BIT = np.tile(bias[None] * Berr, (1, NPIX))        # [B, SW]
    membias = bias[None] * Berr                        # [B, COUT]
    BIT2 = np.zeros((B // 2, TM), np.float32)
    for j in range(2):
        BIT2[:, 32 * j:32 * (j + 1)] = np.tile(membias[j::2], (1, 4))

    import ml_dtypes
    bf16 = ml_dtypes.bfloat16
    XTP = np.zeros((B, 2176, YP), bf16)
    XTP[:, 8:8 + W_IMG * CIN, 1:1 + Y] = \
        X.transpose(0, 2, 3, 1).reshape(B, W_IMG * CIN, Y)
    # main strips: whole 128 rows incl. y-pad columns (keeps each
    # partition row one contiguous 9.3KB DMA run)
    XS = np.empty((B, KP, NSM, YP), bf16)
    for s in range(NSM):
        XS[:, :, s, :] = XTP[:, 112 * s:112 * s + KP, :]
    # tail rows: xi 251..256 -> XT rows 2016..2063, paired samples
    XST = np.ascontiguousarray(
        XTP[:, 2016:2064].reshape(B // 2, 2 * 48, YP))        # [B/2,96,YP]
    return XS, XST, TW.astype(bf16), TWT.astype(bf16), BIT, BIT2


def host_unpack(zt_all, ztt_all):
    out = np.empty((B, Y, W_IMG, COUT), np.float32)
    z = zt_all.reshape(B, NPIX, COUT, NSM, Y)
    z = z.transpose(0, 4, 3, 1, 2).reshape(B, Y, NSM * NPIX, COUT)
    out[:, :, :NSM * NPIX, :] = z
    zt2 = ztt_all.reshape(B // 2, 2, 4, COUT, Y)   # [pair, j, xo, co, y]
    for j in range(2):
        out[j::2, :, NSM * NPIX:, :] = zt2[:, j].transpose(0, 3, 1, 2)
    return out


def kernel(X, W, bias, Werr, Berr):
    from concourse.bass_utils import run_bass_kernel_spmd

    XS, XST, TW, TWT, BIT, BIT2 = host_prep(X, W, bias, Werr, Berr)
    in_maps = []
    for m in range(NCORES):
        sl = slice(m * BPC, (m + 1) * BPC)
        sl2 = slice(m * NP2, (m + 1) * NP2)
        in_maps.append({
            "xs": np.ascontiguousarray(XS[sl]),
            "xst": np.ascontiguousarray(XST[sl2]),
            "tw": np.ascontiguousarray(TW[sl]),
            "twt": np.ascontiguousarray(TWT[sl2]),
            "bi": np.ascontiguousarray(BIT[sl].T),
            "bit": np.ascontiguousarray(BIT2[sl2].T),
        })
    nc = _get_nc()
    res = run_bass_kernel_spmd(nc, in_maps, core_ids=list(range(NCORES)), trace=TRACE)
    LAST_RESULT[0] = res
    zt_all = np.concatenate([r["zt"] for r in res.results], axis=0)
    ztt_all = np.concatenate([r["ztt"] for r in res.results], axis=0)
    return host_unpack(zt_all, ztt_all)
